# revision 46
# baseline (speedup 1.0000x reference)
"""TRN2 Bass kernel for nn_CrossAttention (sparse channel attention + prompt
fusion), sharded spatially over 8 NeuronCores.  Self-contained: builds the
SPMD Bass/Tile program once, shards the full inputs host-side (16 image rows
per core + halo), runs via run_bass_kernel_spmd, and reassembles the output.

Pipelined schedule: per-batch Gram AllReduce overlaps the other batch's
compute; attn@v is folded into the projection (M = W_proj @ blockdiag(A));
q/k pixel-major transposes issue per-subpass so Grams start early.
"""
import sys

for _p in ("/opt/trn_rl_repo", "/root/.axon_site/_ro/trn_rl_repo"):
    if _p not in sys.path:
        sys.path.insert(0, _p)

import numpy as np

B, DIM, HEADS, Himg, Wimg = 2, 384, 8, 128, 128
C = DIM // HEADS            # 48
QKVC = 3 * DIM              # 1152
NCORE = 8
ROWS = Himg // NCORE        # 16 rows per core
NL = ROWS * Wimg            # 2048 local pixels
HR = ROWS + 2               # 18 rows with halo
NH = HR * Wimg              # 2304 halo pixels
PADW = Wimg + 2             # 130
NPAD = HR * PADW            # 2340 padded-free size
NSEG = 3                    # 384 attention rows per batch / 128
BH = B * HEADS              # 16

NO_COLLECTIVE = False       # debug hook: replace AllReduce with local copy


def build_dw_units():
    """Returns (units, perm) where units is a list of dicts and perm maps
    raw qkv channel -> (in_tile m, in_part p).  44 units per batch."""
    units = []
    for kind, koff in (("q", 0), ("k", DIM)):
        for h in range(HEADS):
            units.append(dict(kind=kind, raw_base=koff + C * h, length=32,
                              out_tile=h // 2, out_base=64 * (h % 2),
                              j=2 * (h % 2), half=False))
            units.append(dict(kind=kind, raw_base=koff + C * h + 32, length=16,
                              out_tile=h // 2, out_base=64 * (h % 2) + 32,
                              j=2 * (h % 2) + 1, half=True))
    for t in range(12):
        units.append(dict(kind="v", raw_base=2 * DIM + 32 * t, length=32,
                          out_tile=t // 4, out_base=32 * (t % 4),
                          j=t % 4, half=False))

    def subpass_of(u):
        if u["kind"] == "v":
            return 2
        return 0 if (u["raw_base"] % DIM) < 4 * C else 1

    jc_ctr = {}
    for u in units:
        key = (subpass_of(u), u["j"])
        o = jc_ctr.get(key, 0)
        jc_ctr[key] = o + 1
        u["i"] = (u["j"] + o) % 4 if u["kind"] == "v" else o
    assert all(u["i"] < 4 for u in units)

    slot_next = {}
    half_open = {}
    for u in units:
        spi, i = subpass_of(u), u["i"]
        key = (spi, i)
        if not u["half"]:
            m = 3 * spi + slot_next.get(key, 0)
            slot_next[key] = slot_next.get(key, 0) + 1
            u["in_tile"], u["in_off"] = m, 0
        else:
            if key in half_open:
                u["in_tile"], u["in_off"] = half_open.pop(key), 16
            else:
                m = 3 * spi + slot_next.get(key, 0)
                slot_next[key] = slot_next.get(key, 0) + 1
                half_open[key] = m
                u["in_tile"], u["in_off"] = m, 0
    assert not half_open, half_open

    slot_ctr = {i: 0 for i in range(4)}
    for u in units:
        i = u["i"]
        u["colbase"] = slot_ctr[i] * 9 * 32
        slot_ctr[i] += 1

    perm = np.full((QKVC, 2), -1, np.int64)
    for u in units:
        for r in range(u["length"]):
            raw = u["raw_base"] + r
            p = 32 * u["i"] + u["in_off"] + r
            perm[raw] = (u["in_tile"], p)
    assert (perm >= 0).all()
    return units, perm


def prep_constants(inputs):
    import ml_dtypes
    bf16 = ml_dtypes.bfloat16
    units, perm = build_dw_units()
    w_qkv = np.asarray(inputs["w_qkv"], np.float32)
    w_dw = np.asarray(inputs["w_dw"], np.float32).reshape(QKVC, 9)
    w_proj = np.asarray(inputs["w_proj"], np.float32)

    wqkvT = np.zeros((3, 128, QKVC), np.float32)
    for raw in range(QKVC):
        m, p = perm[raw]
        wqkvT[:, :, m * 128 + p] = w_qkv[raw].reshape(3, 128)

    maxu_per_i = max(sum(1 for u in units if u["i"] == i) for i in range(4))
    COLS = maxu_per_i * 9 * 32
    dwdiag = np.zeros((128, COLS), np.float32)
    for u in units:
        i = u["i"]
        for t in range(9):
            cb = u["colbase"] + t * 32
            for c in range(u["length"]):
                raw = u["raw_base"] + c
                r = u["in_off"] + c
                dwdiag[32 * i + r, cb + c] = w_dw[raw, t]

    def lhsT3(w):   # w [out, in=384] -> [3, 128, out]
        return np.transpose(np.asarray(w, np.float32).reshape(-1, 3, 128), (1, 2, 0)).copy()

    # sel3: [8, 384] f32: sel3[j, 128s+p] = 1 if (128s+p)//48 == j
    sel = np.zeros((HEADS, NSEG * 128), np.float32)
    for r in range(NSEG * 128):
        sel[r // 48, r] = 1.0

    out = dict(
        wqkvT=wqkvT.astype(bf16),
        dwdiag=dwdiag.astype(bf16),
        wprojT=lhsT3(w_proj).astype(bf16),
        w1T_chr=lhsT3(inputs["chr_w1"]).astype(bf16),
        w1T_detg=lhsT3(inputs["detg_w1"]).astype(bf16),
        w2_chr=lhsT3(inputs["chr_w2"]).astype(bf16),     # [3,128,1]
        w2_detg=lhsT3(inputs["detg_w2"]).astype(bf16),
        wtT_chr=np.asarray(inputs["chr_wt"], np.float32).T.copy().astype(bf16),
        wtT_detg=np.asarray(inputs["detg_wt"], np.float32).T.copy().astype(bf16),
        b1_chr=np.ascontiguousarray(np.asarray(inputs["chr_b1"], np.float32).reshape(3, 128).T),
        b1_detg=np.ascontiguousarray(np.asarray(inputs["detg_b1"], np.float32).reshape(3, 128).T),
        bt_chr=np.ascontiguousarray(np.asarray(inputs["chr_bt"], np.float32).reshape(3, 128).T),
        bt_detg=np.ascontiguousarray(np.asarray(inputs["detg_bt"], np.float32).reshape(3, 128).T),
        b2s=np.asarray([float(np.asarray(inputs["chr_b2"]).ravel()[0]),
                        float(np.asarray(inputs["detg_b2"]).ravel()[0])], np.float32),
        temp8=np.asarray(inputs["temperature"], np.float32).reshape(HEADS, 1).copy(),
        attns2=np.asarray(inputs["attns"], np.float32).reshape(1, 2).copy(),
        detg_z=np.asarray(inputs["detg_z"], np.float32).reshape(1, 64).copy(),
        sel3=sel,
        ones1=np.ones((1, 128), np.float32),
    )
    return out, units, perm


def shard_inputs(inputs, consts):
    import ml_dtypes
    bf16 = ml_dtypes.bfloat16
    x = np.asarray(inputs["x"], np.float32)
    gk0 = np.asarray(inputs["gk0"], np.float32)
    gk1 = np.asarray(inputs["gk1"], np.float32)
    xp = np.pad(x, ((0, 0), (0, 0), (1, 1), (0, 0)))
    maps = []
    for ci in range(NCORE):
        r0 = ROWS * ci
        xs = xp[:, :, r0:r0 + HR, :].reshape(B, 3, 128, NH)
        g0 = gk0[:, :, r0:r0 + ROWS, :].reshape(B, 64, NL)
        g1 = gk1[:, :, r0:r0 + ROWS, :].reshape(B, 64, NL)
        m = {"x_s": np.ascontiguousarray(xs).astype(bf16),
             "gk0_s": np.ascontiguousarray(g0).astype(bf16),
             "gk1_s": np.ascontiguousarray(g1).astype(bf16)}
        m.update({k: v for k, v in consts.items()})
        maps.append(m)
    return maps


from contextlib import ExitStack

import concourse.bass as bass
import concourse.tile as tile
import concourse.mybir as mybir
from concourse import bacc

f32 = mybir.dt.float32
bf16 = mybir.dt.bfloat16
AX = mybir.AxisListType
OP = mybir.AluOpType
AF = mybir.ActivationFunctionType
CH = 512          # pixel chunk for most matmuls
NCHUNK = NL // CH  # 4


def build_program():
    units, _ = build_dw_units()
    maxu = max(sum(1 for u in units if u["i"] == i) for i in range(4))
    DWCOLS = maxu * 9 * 32

    nc = bacc.Bacc("TRN2", debug=False, num_devices=NCORE,
                   target_bir_lowering=False)

    def din(name, shape, dt=bf16):
        return nc.dram_tensor(name, list(shape), dt, kind="ExternalInput").ap()

    t = {}
    t["x_s"] = din("x_s", (B, 3, 128, NH))
    t["gk0_s"] = din("gk0_s", (B, 64, NL))
    t["gk1_s"] = din("gk1_s", (B, 64, NL))
    t["wqkvT_d"] = din("wqkvT", (3, 128, QKVC))
    t["dwdiag_d"] = din("dwdiag", (128, DWCOLS))
    t["wprojT_d"] = din("wprojT", (3, 128, DIM))
    t["w1T_d"] = {"chr": din("w1T_chr", (3, 128, DIM)), "detg": din("w1T_detg", (3, 128, DIM))}
    t["w2_d"] = {"chr": din("w2_chr", (3, 128, 1)), "detg": din("w2_detg", (3, 128, 1))}
    t["wtT_d"] = {"chr": din("wtT_chr", (64, DIM)), "detg": din("wtT_detg", (64, DIM))}
    t["b1_d"] = {"chr": din("b1_chr", (128, 3), f32), "detg": din("b1_detg", (128, 3), f32)}
    t["bt_d"] = {"chr": din("bt_chr", (128, 3), f32), "detg": din("bt_detg", (128, 3), f32)}
    t["b2s_d"] = din("b2s", (1, 2), f32)
    t["temp_d"] = din("temp8", (HEADS, 1), f32)
    t["attns_d"] = din("attns2", (1, 2), f32)
    t["zrow_d"] = din("detg_z", (1, 64), f32)

    t["OUT"] = nc.dram_tensor("OUT", [B, 3, 128, NL], bf16, kind="ExternalOutput").ap()

    # internal DRAM
    t["g3_part"] = nc.dram_tensor("g3_part", [BH, 96, 96], f32)
    t["g3_all"] = nc.dram_tensor("g3_all", [BH, 96, 96], f32, addr_space="Shared")
    t["qk_dense"] = [nc.dram_tensor(f"qk_dense{b}", [NSEG * 128, 48], f32) for b in range(B)]
    t["rq_flat"] = [nc.dram_tensor(f"rq_flat{b}", [NSEG * 128], f32) for b in range(B)]
    t["a_dram"] = [nc.dram_tensor(f"a_dram{b}", [NSEG * 128 * 48], bf16) for b in range(B)]
    t["rk_dram"] = [nc.dram_tensor(f"rk_dram{b}", [HEADS, 48], f32) for b in range(B)]
    t["zb_d"] = nc.dram_tensor("zb", [64], f32)

    with tile.TileContext(nc) as tc, ExitStack() as ctx:
        _body(tc, ctx, units, t)
    nc.compile()
    return nc


def _body(tc, ctx, units, t):
    nc = tc.nc
    ec = [0]

    def ecopy(out_ap, in_ap):
        if ec[0] % 2 == 0:
            nc.scalar.copy(out_ap, in_ap)
        else:
            nc.vector.tensor_copy(out_ap, in_ap)
        ec[0] += 1

    wp = ctx.enter_context(tc.tile_pool(name="wp", bufs=1))
    pp = ctx.enter_context(tc.tile_pool(name="pp", bufs=1))
    sp = ctx.enter_context(tc.tile_pool(name="sp", bufs=1))
    ps_pool = ctx.enter_context(tc.tile_pool(name="ps", bufs=2, space="PSUM"))

    # ---------------- stage-0 DMAs: wqkv[0], x(b0), rest ----------------
    x_sb = [pp.tile([128, NH], bf16, tag=f"x{k}", name=f"x{k}") for k in range(3)]
    wqkvT = [wp.tile([128, QKVC], bf16, tag=f"wqkv{k}", name=f"wqkv{k}") for k in range(3)]
    nc.scalar.dma_start(wqkvT[0][:], t["wqkvT_d"][0])
    nc.sync.dma_start(x_sb[0][:], t["x_s"][0, 0])
    nc.scalar.dma_start(x_sb[1][:], t["x_s"][0, 1])
    nc.sync.dma_start(x_sb[2][:], t["x_s"][0, 2])
    for k in range(1, 3):
        nc.scalar.dma_start(wqkvT[k][:], t["wqkvT_d"][k])
    dwdiag = wp.tile([128, t["dwdiag_d"].shape[1]], bf16, tag="dwdiag", name="dwdiag")
    nc.scalar.dma_start(dwdiag[:], t["dwdiag_d"][:])

    # late-needed consts (declared now, loaded in deferred_consts())
    wprojT = [wp.tile([128, DIM], bf16, tag=f"wproj{k}", name=f"wproj{k}") for k in range(3)]
    w1T, w2, wtT, b1, bt = {}, {}, {}, {}, {}
    for br in ("chr", "detg"):
        w1T[br] = [wp.tile([128, DIM], bf16, tag=f"w1{br}{k}", name=f"w1{br}{k}") for k in range(3)]
        w2[br] = [wp.tile([128, 1], bf16, tag=f"w2{br}{k}", name=f"w2{br}{k}") for k in range(3)]
        wtT[br] = wp.tile([64, DIM], bf16, tag=f"wt{br}", name=f"wt{br}")
        b1[br] = wp.tile([128, 3], f32, tag=f"b1{br}", name=f"b1{br}")
        bt[br] = wp.tile([128, 3], f32, tag=f"bt{br}", name=f"bt{br}")
    b2sb = wp.tile([1, 2], f32, tag="b2", name="b2")
    tempc = wp.tile([HEADS, 1], f32, tag="temp", name="temp")
    zrow = wp.tile([1, 64], f32, tag="zrow", name="zrow")
    ones64f = wp.tile([64, 128], f32, tag="ones64f", name="ones64f")
    ones1b = wp.tile([1, 64], bf16, tag="ones1b", name="ones1b")
    attns_bc = wp.tile([128, 2], f32, tag="attnsbc", name="attnsbc")
    zrep16 = wp.tile([64, 128], bf16, tag="zrep16", name="zrep16")

    def deferred_consts():
        for k in range(3):
            nc.gpsimd.dma_start(wprojT[k][:], t["wprojT_d"][k])
        for br in ("chr", "detg"):
            for k in range(3):
                nc.gpsimd.dma_start(w1T[br][k][:], t["w1T_d"][br][k])
                nc.gpsimd.dma_start(w2[br][k][:], t["w2_d"][br][k])
            nc.gpsimd.dma_start(wtT[br][:], t["wtT_d"][br][:])
            nc.gpsimd.dma_start(b1[br][:], t["b1_d"][br][:])
            nc.gpsimd.dma_start(bt[br][:], t["bt_d"][br][:])
        nc.gpsimd.dma_start(b2sb[:], t["b2s_d"][:])
        nc.gpsimd.dma_start(tempc[:], t["temp_d"][:])
        nc.gpsimd.dma_start(zrow[:], t["zrow_d"][:])
        nc.vector.memset(ones64f[:], 1.0)
        nc.vector.memset(ones1b[:], 1.0)
        # attns broadcast to all partitions via stride-0 DMA
        nc.gpsimd.dma_start(attns_bc[:],
                            bass.AP(tensor=t["attns_d"].tensor, offset=0,
                                    ap=[[0, 128], [1, 2]]))
        # z-bar prep: z / max(||z||, 1e-12), replicated [64, 128] bf16
        zsq = sp.tile([1, 64], f32, tag="zsq", name="zsq")
        nc.scalar.square(zsq[:], zrow[:])
        zss = sp.tile([1, 1], f32, tag="zss", name="zss")
        nc.vector.reduce_sum(zss[:], zsq[:], axis=AX.X)
        nc.scalar.sqrt(zss[:], zss[:])
        nc.vector.tensor_scalar_max(zss[:], zss[:], 1e-12)
        zrs = sp.tile([1, 1], f32, tag="zrs", name="zrs")
        nc.vector.reciprocal(zrs[:], zss[:])
        zn = sp.tile([1, 64], f32, tag="zn", name="zn")
        nc.vector.tensor_scalar_mul(zn[:], zrow[:], zrs[:, 0:1])
        nc.sync.dma_start(t["zb_d"].ap().rearrange("(a b) -> a b", a=1), zn[:])
        zcol = sp.tile([64, 1], f32, tag="zcol", name="zcol")
        nc.sync.dma_start(zcol[:], t["zb_d"].ap().rearrange("(p a) -> p a", a=1))
        zrep = sp.tile([64, 128], f32, tag="zrep", name="zrep")
        nc.vector.tensor_scalar_mul(zrep[:], ones64f[:], zcol[:, 0:1])
        nc.vector.tensor_copy(zrep16[:], zrep[:])

    # persistent per-batch tensors
    vcm = [[wp.tile([128, NL], bf16, tag=f"v{b}_{mv}", name=f"v{b}_{mv}") for mv in range(3)]
           for b in range(B)]

    units_by_sp = [
        [u for u in units if u["kind"] in "qk" and u["raw_base"] % DIM < 4 * C],
        [u for u in units if u["kind"] in "qk" and u["raw_base"] % DIM >= 4 * C],
        [u for u in units if u["kind"] == "v"],
    ]

    qpad = [pp.tile([128, NL], bf16, tag=f"big4k_{i2}", name=f"qpad{i2}") for i2 in range(4)]
    kpad = [pp.tile([128, NL], bf16, tag=f"big4k_{4 + i2}", name=f"kpad{i2}") for i2 in range(4)]
    s_pm = pp.tile([128, 16 * 768], bf16, tag="spm", name="s_pm")
    spm3 = s_pm[:].rearrange("p (c blk) -> p c blk", blk=768)

    tp_ctr = [0]

    def emit_transposes(heads):
        # ~1.7us fixed cost per call on the issuing ring: alternate rings so
        # eight calls serialize at ~7us instead of ~14us
        for h in heads:
            for qk, koff in ((qpad, 0), (kpad, 48)):
                src = qk[h // 2][64 * (h % 2):64 * (h % 2) + 48, :]
                eng = nc.sync if tp_ctr[0] % 2 == 0 else nc.scalar
                tp_ctr[0] += 1
                eng.dma_start_transpose(
                    spm3[:, :, 96 * h + koff: 96 * h + koff + 48], src)

    def emit_qkv_dw(b):
        """qkv conv + dw conv for batch b; transposes per q/k subpass."""
        for spi, us in enumerate(units_by_sp):
            qkvpad = pp.tile([128, 3 * NPAD], bf16, tag="bigA", name="qkvpad")
            for mg in range(3):
                m = 3 * spi + mg
                pv = qkvpad[:, mg * NPAD:(mg + 1) * NPAD].rearrange(
                    "p (r w) -> p r w", w=PADW)
                nc.vector.memset(pv[:, :, 0:1], 0.0)
                nc.vector.memset(pv[:, :, PADW - 1:PADW], 0.0)
                for nck in range(6):           # 6 x 384-pixel chunks (3 rows)
                    psq = ps_pool.tile([128, 384], f32, tag=f"dw{nck % 4}", name="qkvps")
                    for k in range(3):
                        nc.tensor.matmul(
                            psq[:], wqkvT[k][:, m * 128:(m + 1) * 128],
                            x_sb[k][:, nck * 384:(nck + 1) * 384],
                            start=(k == 0), stop=(k == 2))
                    ecopy(pv[:, 3 * nck:3 * nck + 3, 1:129],
                          psq[:].rearrange("p (r w) -> p r w", w=128))

            outkeys = sorted({(u["kind"], u["out_tile"]) for u in us})
            for ck in range(NCHUNK):
                pso = {ok: ps_pool.tile([128, CH], f32, tag=f"dw{oi}", name=f"dw{ok[0]}{ok[1]}")
                       for oi, ok in enumerate(outkeys)}
                for tap in range(9):
                    dy, dx = tap // 3, tap % 3
                    for u in us:
                        mg = u["in_tile"] - 3 * spi
                        src = qkvpad[32 * u["i"]:32 * u["i"] + 32,
                                     mg * NPAD:(mg + 1) * NPAD]
                        rhs = src.rearrange("p (r w) -> p r w", w=PADW)[
                            :, 4 * ck + dy: 4 * ck + dy + 4, dx:dx + 128]
                        lhsT = dwdiag[32 * u["i"]:32 * u["i"] + 32,
                                      u["colbase"] + tap * 32: u["colbase"] + tap * 32 + 32]
                        ob = u["out_base"]
                        out = pso[(u["kind"], u["out_tile"])][ob:ob + 32, :]
                        nc.tensor.matmul(out, lhsT, rhs,
                                         start=(tap == 0), stop=(tap == 8),
                                         tile_position=(32 * u["i"], ob))
                for (kind, ot), ps in pso.items():
                    dst = {"q": qpad, "k": kpad, "v": vcm[b]}[kind][ot]
                    if kind == "v":
                        ecopy(dst[:, ck * CH:(ck + 1) * CH], ps[:])
                    else:
                        for pb in (0, 64):
                            ecopy(dst[pb:pb + 48, ck * CH:(ck + 1) * CH],
                                  ps[pb:pb + 48, :])
            if spi == 0:
                emit_transposes(range(0, 4))
            elif spi == 1:
                emit_transposes(range(4, 8))

    def emit_gram(b):
        g3sb = pp.tile([96, 8 * 96], f32, tag="g3sb", name="g3sb")
        for h in range(HEADS):
            psg = ps_pool.tile([96, 96], f32, tag=f"dw{h % 4}", name="g3ps")
            for ckk in range(16):
                lhs = spm3[:, ckk, 96 * h:96 * h + 96]
                nc.tensor.matmul(psg[:], lhs, lhs,
                                 start=(ckk == 0), stop=(ckk == 15))
            nc.vector.tensor_copy(g3sb[:, 96 * h:96 * (h + 1)], psg[:])
        nc.sync.dma_start(
            t["g3_part"].ap()[8 * b:8 * b + 8].rearrange("h r c -> r h c"),
            g3sb[:].rearrange("r (h c) -> r h c", c=96))

    def emit_allreduce(b):
        if NO_COLLECTIVE:
            nc.sync.dma_start(t["g3_all"].ap()[8 * b:8 * b + 8],
                              t["g3_part"].ap()[8 * b:8 * b + 8])
        else:
            nc.gpsimd.collective_compute(
                "AllReduce", OP.add, replica_groups=[list(range(NCORE))],
                ins=[t["g3_part"].ap()[8 * b:8 * b + 8].opt()],
                outs=[t["g3_all"].ap()[8 * b:8 * b + 8].opt()])

    def emit_attn(b, gate_src):
        """Attention matrices for batch b: [128, 3*48] seg layout -> a_dram[b].

        gate_src: a tiny AP written late in the preceding phase; a dummy
        copy into norm2 makes every op in this chain transitively depend on
        it, so the Tile scheduler cannot hoist AR-dependent ops into the
        middle of busy engine queues (the static scheduler models the
        collective as fast; at runtime a hoisted op head-of-line blocks its
        FIFO for the collective's full latency).
        """
        g3a = t["g3_all"]
        base = 8 * b * 96 * 96
        norm2 = sp.tile([HEADS, 96], f32, tag="norm2", name="norm2")
        nc.vector.tensor_copy(norm2[0:1, 0:1], gate_src)
        nc.sync.dma_start(norm2[:].rearrange("h (q d) -> h q d", q=2),
                          bass.AP(tensor=g3a, offset=base,
                                  ap=[[96 * 96, HEADS], [48 * 96 + 48, 2], [97, 48]]))
        nc.scalar.sqrt(norm2[:], norm2[:])
        nc.vector.tensor_scalar_max(norm2[:], norm2[:], 1e-12)
        rn = sp.tile([HEADS, 96], f32, tag="rn", name="rn")
        nc.vector.reciprocal(rn[:], norm2[:])
        rqf = sp.tile([HEADS, 48], f32, tag="rqf", name="rqf")
        nc.vector.tensor_scalar_mul(rqf[:], rn[:, 0:48], tempc[:, 0:1])
        nc.sync.dma_start(t["rq_flat"][b].ap().rearrange("(a c) -> a c", a=HEADS), rqf[:])
        rq_seg = sp.tile([128, NSEG], f32, tag="rqseg", name="rqseg")
        nc.sync.dma_start(rq_seg[:],
                          t["rq_flat"][b].ap().rearrange("(s p) -> p s", s=NSEG))
        # rk broadcast [8,48] -> [128, 3*48] via stride-0 DMAs (no PE use,
        # so the whole attn chain runs while the PE is busy elsewhere)
        nc.sync.dma_start(t["rk_dram"][b].ap(), rn[:, 48:96])
        rk_bc = sp.tile([128, NSEG * 48], f32, tag="rkbc", name="rkbc")
        for s in range(NSEG):
            p = 0
            while p < 128:
                r = 128 * s + p
                h = r // 48
                ln = min(48 * (h + 1) - r, 128 - p)
                nc.sync.dma_start(
                    rk_bc[p:p + ln, 48 * s:48 * s + 48],
                    bass.AP(tensor=t["rk_dram"][b], offset=h * 48,
                            ap=[[0, ln], [1, 48]]))
                p += ln
        nc.sync.dma_start(
            t["qk_dense"][b].ap().rearrange("(h c) d -> h c d", h=HEADS),
            g3a.ap()[8 * b:8 * b + 8, 0:48, 48:96])
        G_seg = sp.tile([128, NSEG * 48], f32, tag="gseg", name="gseg")
        nc.sync.dma_start(G_seg[:].rearrange("p (s d) -> p s d", s=NSEG),
                          t["qk_dense"][b].ap().rearrange("(s p) d -> p s d", s=NSEG))

        A = sp.tile([128, NSEG * 48], f32, tag="A", name="A")
        seg = lambda tl, s: tl[:, 48 * s:48 * s + 48]
        for s in range(NSEG):
            nc.vector.scalar_tensor_tensor(
                out=seg(A, s), in0=seg(G_seg, s), scalar=rq_seg[:, s:s + 1],
                in1=seg(rk_bc, s), op0=OP.mult, op1=OP.mult)

        m1 = sp.tile([128, NSEG * 8], f32, tag="m1", name="m1")
        m2 = sp.tile([128, NSEG * 8], f32, tag="m2", name="m2")
        m3 = sp.tile([128, NSEG * 8], f32, tag="m3", name="m3")
        At1 = sp.tile([128, NSEG * 48], f32, tag="At1", name="At1")
        At2 = sp.tile([128, NSEG * 48], f32, tag="At2", name="At2")
        for s in range(NSEG):
            nc.vector.max(m1[:, 8 * s:8 * s + 8], seg(A, s))
            nc.vector.match_replace(seg(At1, s), m1[:, 8 * s:8 * s + 8], seg(A, s), -1e30)
            nc.vector.max(m2[:, 8 * s:8 * s + 8], seg(At1, s))
            nc.vector.match_replace(seg(At2, s), m2[:, 8 * s:8 * s + 8], seg(At1, s), -1e30)
            nc.vector.max(m3[:, 8 * s:8 * s + 8], seg(At2, s))

        rowst = sp.tile([128, NSEG], f32, tag="rowst", name="rowst")   # -rowmax
        nc.vector.reduce_max(rowst[:], m1[:].rearrange("p (s e) -> p s e", e=8), axis=AX.X)
        nc.vector.tensor_scalar_mul(rowst[:], rowst[:], -1.0)
        t24 = sp.tile([128, NSEG], f32, tag="t24", name="t24")
        nc.vector.tensor_reduce(t24[:], m3[:].rearrange("p (s e) -> p s e", e=8),
                                axis=AX.X, op=OP.min)
        t12 = sp.tile([128, NSEG], f32, tag="t12", name="t12")
        m2v = m2[:].rearrange("p (s e) -> p s e", e=8)
        nc.vector.tensor_copy(t12[:], m2v[:, :, 3])

        e1 = sp.tile([128, NSEG * 48], f32, tag="e1", name="e1")
        p1 = sp.tile([128, NSEG * 48], f32, tag="p1", name="p1")
        Z1 = sp.tile([128, NSEG], f32, tag="Z1", name="Z1")
        for s in range(NSEG):
            nc.scalar.activation(seg(e1, s), seg(A, s), AF.Exp,
                                 bias=rowst[:, s:s + 1], scale=1.0)
            nc.vector.scalar_tensor_tensor(
                out=seg(p1, s), in0=seg(A, s), scalar=t24[:, s:s + 1],
                in1=seg(e1, s), op0=OP.is_ge, op1=OP.mult,
                accum_out=Z1[:, s:s + 1])
        r1 = sp.tile([128, NSEG], f32, tag="r1", name="r1")
        nc.vector.reciprocal(r1[:], Z1[:])
        e2 = At2  # At2 is dead once m3 is built; reuse its buffer
        p2 = sp.tile([128, NSEG * 48], f32, tag="p2", name="p2")
        Z2 = sp.tile([128, NSEG], f32, tag="Z2", name="Z2")
        for s in range(NSEG):
            nc.scalar.activation(seg(e2, s), seg(p1, s), AF.Exp,
                                 bias=0.0, scale=r1[:, s:s + 1])
            nc.vector.scalar_tensor_tensor(
                out=seg(p2, s), in0=seg(A, s), scalar=t12[:, s:s + 1],
                in1=seg(e2, s), op0=OP.is_ge, op1=OP.mult,
                accum_out=Z2[:, s:s + 1])
        r2 = sp.tile([128, NSEG], f32, tag="r2", name="r2")
        nc.vector.reciprocal(r2[:], Z2[:])
        r1p = sp.tile([128, NSEG], f32, tag="r1p", name="r1p")
        nc.vector.tensor_scalar_mul(r1p[:], r1[:], attns_bc[:, 0:1])
        r2p = sp.tile([128, NSEG], f32, tag="r2p", name="r2p")
        nc.vector.tensor_scalar_mul(r2p[:], r2[:], attns_bc[:, 1:2])

        ac = sp.tile([128, NSEG * 48], f32, tag="ac", name="ac")
        tmpc = e1  # e1 is dead once p1 is built; reuse its buffer
        for s in range(NSEG):
            nc.vector.tensor_scalar_mul(seg(tmpc, s), seg(p2, s), r2p[:, s:s + 1])
            nc.vector.scalar_tensor_tensor(
                out=seg(ac, s), in0=seg(p1, s), scalar=r1p[:, s:s + 1],
                in1=seg(tmpc, s), op0=OP.mult, op1=OP.add)
        acb = sp.tile([128, NSEG * 48], bf16, tag="acb", name="acb")
        nc.vector.tensor_copy(acb[:], ac[:])
        nc.sync.dma_start(
            t["a_dram"][b].ap().rearrange("(s p d) -> p s d", s=NSEG, p=128),
            acb[:].rearrange("p (s d) -> p s d", s=NSEG))

    abd = [pp.tile([128, DIM], bf16, tag=f"abd{j}", name=f"abd{j}") for j in range(3)]
    Mt = [pp.tile([128, DIM], bf16, tag=f"Mt{k}", name=f"Mt{k}") for k in range(3)]
    out0 = [pp.tile([128, NL], bf16, tag=f"big4k_{3 + mo}", name=f"out0{mo}") for mo in range(3)]
    prom_chr = [pp.tile([128, NL], bf16, tag=f"pr{mo}", name=f"prchr{mo}") for mo in range(3)]
    oh = {}

    def emit_out_head(b):
        """M = Wproj @ blockdiag(A); out0 = M @ v; chr prompt branch."""
        adr = t["a_dram"][b].ap().rearrange("(r d) -> r d", d=48)
        for j in range(3):
            nc.vector.memset(abd[j][:], 0.0)
        for h in range(HEADS):
            for j in range(3):
                d0 = max(0, 128 * j - 48 * h)
                d1 = min(48, 128 * j + 128 - 48 * h)
                if d0 >= d1:
                    continue
                p0 = 48 * h + d0 - 128 * j
                nc.scalar.dma_start(abd[j][p0:p0 + (d1 - d0), 48 * h:48 * h + 48],
                                    adr[48 * h + d0:48 * h + d1, :])
        for k in range(3):
            pm = ps_pool.tile([128, DIM], f32, tag=f"dw{k}", name="Mps")
            for j in range(3):
                nc.tensor.matmul(pm[:], abd[j][:, 128 * k:128 * k + 128],
                                 wprojT[j][:], start=(j == 0), stop=(j == 2))
            nc.vector.tensor_copy(Mt[k][:], pm[:])

        # out0 = M @ v
        for mo in range(3):
            for ck in range(NCHUNK):
                psp = ps_pool.tile([128, CH], f32, tag=f"dw{ck % 4}", name="projps")
                for k in range(3):
                    nc.tensor.matmul(psp[:], Mt[k][:, 128 * mo:128 * mo + 128],
                                     vcm[b][k][:, ck * CH:(ck + 1) * CH],
                                     start=(k == 0), stop=(k == 2))
                ecopy(out0[mo][:, ck * CH:(ck + 1) * CH], psp[:])

        # gk loads + alpha (own tags: sharing kpad tags would make these
        # loads wait on the other batch's transposes and HOL-block the ring)
        gk_sb = {}
        for br, gk_d in (("chr", t["gk0_s"]), ("detg", t["gk1_s"])):
            gk_sb[br] = pp.tile([64, NL], bf16, tag=f"gk{br}", name=f"gk{br}")
            nc.gpsimd.dma_start(gk_sb[br][:], gk_d[b])
        al16 = pp.tile([128, NL], bf16, tag="al16", name="al16")
        for ck in range(NCHUNK):
            psal = ps_pool.tile([128, CH], f32, tag=f"dw{ck % 4}", name="alps")
            nc.tensor.matmul(psal[:], zrep16[:], gk_sb["detg"][:, ck * CH:(ck + 1) * CH],
                             start=True, stop=True)
            nc.scalar.copy(al16[:, ck * CH:(ck + 1) * CH], psal[:])

        oh["gk"] = gk_sb
        oh["al16"] = al16
        branch(b, "chr", store_chr)
        # precompute e = out0 + (1-alpha)*chr while PE runs the detg branch;
        # the post-PE drain then needs only 2 DVE ops per chunk
        eb = [pp.tile([128, NL], bf16, tag=f"eb{mo}", name=f"eb{mo}") for mo in range(3)]
        for mo in range(3):
            nc.vector.tensor_mul(eb[mo][:], al16[:], prom_chr[mo][:])
            nc.vector.tensor_sub(eb[mo][:], prom_chr[mo][:], eb[mo][:])
            nc.vector.tensor_add(eb[mo][:], eb[mo][:], out0[mo][:])
        oh["eb"] = eb

    def branch(b, br, store):
        gk_sb = oh["gk"]
        g16 = [pp.tile([128, NL], bf16, tag=f"big4k_{mo}", name=f"g16{mo}") for mo in range(3)]
        for mo in range(3):
            for ck in range(NCHUNK):
                psg = ps_pool.tile([128, CH], f32, tag=f"dw{ck % 4}", name="gps")
                for k in range(3):
                    nc.tensor.matmul(psg[:], w1T[br][k][:, 128 * mo:128 * mo + 128],
                                     out0[k][:, ck * CH:(ck + 1) * CH],
                                     start=(k == 0), stop=(k == 2))
                nc.scalar.activation(g16[mo][:, ck * CH:(ck + 1) * CH], psg[:],
                                     AF.Gelu, bias=b1[br][:, mo:mo + 1], scale=1.0)
        gate16 = pp.tile([1, NL], bf16, tag="x0g", name="gate16")
        for ck in range(NCHUNK):
            psgt = ps_pool.tile([1, CH], f32, tag=f"dw{ck % 4}", name="gateps")
            for k in range(3):
                nc.tensor.matmul(psgt[:], w2[br][k][:],
                                 g16[k][:, ck * CH:(ck + 1) * CH],
                                 start=(k == 0), stop=(k == 2))
            bi = 0 if br == "chr" else 1
            nc.scalar.activation(gate16[:, ck * CH:(ck + 1) * CH], psgt[:],
                                 AF.Sigmoid, bias=b2sb[0:1, bi:bi + 1], scale=1.0)
        gated16 = pp.tile([64, NL], bf16, tag="x1g", name="gated16")
        for ck in range(NCHUNK):
            psgb = ps_pool.tile([64, CH], f32, tag=f"dw{(ck + 2) % 4}", name="gbps")
            nc.tensor.matmul(psgb[:], ones1b[:], gate16[0:1, ck * CH:(ck + 1) * CH],
                             start=True, stop=True)
            nc.vector.scalar_tensor_tensor(
                out=gated16[:, ck * CH:(ck + 1) * CH], in0=gk_sb[br][:, ck * CH:(ck + 1) * CH],
                scalar=1.0, in1=psgb[:], op0=OP.mult, op1=OP.mult)
        for mo in range(3):
            for ck in range(NCHUNK):
                pst2 = ps_pool.tile([128, CH], f32, tag=f"dw{ck % 4}", name="transps")
                nc.tensor.matmul(pst2[:], wtT[br][:, 128 * mo:128 * mo + 128],
                                 gated16[:, ck * CH:(ck + 1) * CH],
                                 start=True, stop=True)
                store(b, mo, ck, pst2)

    def store_chr(b, mo, ck, pst2):
        nc.scalar.activation(prom_chr[mo][:, ck * CH:(ck + 1) * CH], pst2[:],
                             AF.Silu, bias=bt["chr"][:, mo:mo + 1], scale=1.0)

    prd_hold = {}

    def store_detg(b, mo, ck, pst2):
        # silu per 512-psum chunk into a 1024-wide prd; drain (mul + f32
        # fused add + store) once per 1024 pair: fin = e + alpha*detg with
        # e = out0 + (1-alpha)*chr precomputed
        al16 = oh["al16"]
        if ck % 2 == 0:
            prd_hold["t"] = sp.tile([128, 1024], bf16, tag="prd", name="prd", bufs=3)
        prd = prd_hold["t"]
        nc.scalar.activation(prd[:, (ck % 2) * CH:(ck % 2 + 1) * CH], pst2[:],
                             AF.Silu, bias=bt["detg"][:, mo:mo + 1], scale=1.0)
        if ck % 2 == 0:
            return
        sl = slice((ck - 1) * CH, (ck + 1) * CH)
        t2 = sp.tile([128, 1024], bf16, tag="bl2", name="bl2", bufs=3)
        nc.vector.tensor_mul(t2[:], prd[:], al16[:, sl])
        fin = sp.tile([128, 1024], bf16, tag="fin", name="fin", bufs=2)
        nc.vector.scalar_tensor_tensor(
            out=fin[:], in0=t2[:], scalar=1.0, in1=oh["eb"][mo][:, sl],
            op0=OP.mult, op1=OP.add)
        nc.sync.dma_start(t["OUT"][b, mo][:, sl], fin[:])

    def emit_out_tail(b):
        branch(b, "detg", store_detg)

    # ================= schedule =================
    emit_qkv_dw(0)
    # x for batch 1 + deferred consts while b0 dw finishes
    nc.sync.dma_start(x_sb[0][:], t["x_s"][1, 0])
    nc.scalar.dma_start(x_sb[1][:], t["x_s"][1, 1])
    nc.sync.dma_start(x_sb[2][:], t["x_s"][1, 2])
    deferred_consts()
    emit_gram(0)
    emit_allreduce(0)
    emit_qkv_dw(1)
    # attn(0) gated on the last of b1's transposes (via s_pm tail element):
    # at that point AR0 is long done, so no engine queue HOL-blocks on it;
    # the chain is PE-free (rk via broadcast DMAs) and overlaps b1's tail
    emit_attn(0, s_pm[0:1, 16 * 768 - 1:16 * 768])
    emit_gram(1)
    emit_allreduce(1)
    emit_out_head(0)      # M0, out0, chr branch; AR1 completes underneath
    # attn(1) gated on b0's out0 (early in out(0)); AR1 is done by then.
    # Emitted before tail(0) so PE rolls from b0's wt matmuls straight into
    # M1/out0(1) while b0's blend drains on Act/DVE/gpsimd.
    emit_attn(1, out0[2][0:1, NL - 1:NL])
    emit_out_tail(0)
    emit_out_head(1)
    emit_out_tail(1)


_PROG = None


def _program():
    global _PROG
    if _PROG is None:
        _PROG = build_program()
    return _PROG


def kernel(**inputs):
    from concourse.bass_utils import run_bass_kernel_spmd
    nc = _program()
    consts, _, _ = prep_constants(inputs)
    maps = shard_inputs(inputs, consts)
    res = run_bass_kernel_spmd(nc, maps, list(range(NCORE)))
    out = np.empty((B, DIM, Himg, Wimg), np.float32)
    for ci in range(NCORE):
        o = np.asarray(res.results[ci]["OUT"], np.float32).reshape(B, DIM, ROWS, Wimg)
        out[:, :, ROWS * ci:ROWS * (ci + 1), :] = o
    return out
